# revision 4
# baseline (speedup 1.0000x reference)
"""KVEmbedding lookup v2: host-sorted indices + InstDMAGatherAnt.

Strategy (the sharding_hint's all-to-all routing, with the host as the
network): indices are batch-sharded 8 ways; each core's 409600 lookups are
sorted by value on the host and bucketed into 31 windows of 32768 table rows
(int16 local offsets, the dma_gather addressing reach).  The device runs
14-sub-gather batches (1024 idxs each — the HW per-instruction limit) per
window: ~434 dma_gather instructions/core instead of 3200 tiny 128-row
indirect DMAs, so the ~1us/instruction SWDGE overhead drops 7x and the random
256B HBM reads become ascending (row-buffer friendly).  Gathered rows land in
sorted order; the host inverts the permutation during unshard.

Device per core: per window: 14x dma_gather(table[s*32K:(s+1)*32K]) -> SBUF
stage slot -> contiguous store to DRAM.  Double-buffered stages; gathers on
the Pool SWDGE queue, idx load + stores on the sync HWDGE queue.
"""

from contextlib import ExitStack

import numpy as np

BATCH, HIST = 16384, 200
VOCAB, D = 1_000_000, 64
NCORES = 8
ROWS_PER_CORE = BATCH // NCORES          # 2048
FLAT = ROWS_PER_CORE * HIST              # 409600 lookups per core
P = 128
SEG = 32768                               # int16-addressable window
NSEG = (VOCAB + SEG - 1) // SEG           # 31
SUB = 1024                                # max idxs per dma_gather on HW
CAP = 14336                               # slots per (core, window) bucket
NSUB = CAP // SUB                         # 14 sub-gathers per window
SLOTS = CAP // P                          # stage cols per window (112)

_built = {}


def _build(cap):
    import concourse.bacc as bacc
    import concourse.mybir as mybir
    from concourse._compat import get_trn_type

    nsub, slots = cap // SUB, cap // P
    sub_slots = SUB // P                  # 8 stage cols per sub-gather
    cw = cap // 16                        # idx16 cols per window
    nc = bacc.Bacc(
        get_trn_type() or "TRN2", num_swdge_queues=4,
        dynamic_dma_scratch_size=65536,
    )
    table = nc.declare_dram_parameter(
        "table", [VOCAB, D], mybir.dt.float32, isOutput=False
    )
    idx16 = nc.declare_dram_parameter(
        "idx16", [P, NSEG * cw], mybir.dt.int16, isOutput=False
    )
    out = nc.declare_dram_parameter(
        "out", [NSEG * cap, D], mybir.dt.float32, isOutput=True
    )

    with ExitStack() as ctx:
        t_idx = ctx.enter_context(
            nc.sbuf_tensor("t_idx", [P, NSEG * cw], mybir.dt.int16)
        )
        stage = [
            ctx.enter_context(
                nc.sbuf_tensor(f"stage{b}", [P, slots * D], mybir.dt.float32)
            )
            for b in range(2)
        ]
        ls = ctx.enter_context(nc.semaphore("ls"))
        gsA = [ctx.enter_context(nc.semaphore(f"gsA{b}")) for b in range(2)]
        gsB = [ctx.enter_context(nc.semaphore(f"gsB{b}")) for b in range(2)]
        ssA = [ctx.enter_context(nc.semaphore(f"ssA{b}")) for b in range(2)]
        ssB = [ctx.enter_context(nc.semaphore(f"ssB{b}")) for b in range(2)]
        half = (nsub // 2) * sub_slots * D   # stage cols in half A
        block = ctx.enter_context(nc.Block())

        @block.sync
        def _(sync):
            sync.dma_start(out=t_idx[:], in_=idx16[:, :]).then_inc(ls, 16)
            for s in range(NSEG):
                b, c = s % 2, s // 2
                ow = out[s * cap : (s + 1) * cap, :].rearrange(
                    "(p c) d -> p (c d)", p=P
                )
                sync.wait_ge(gsA[b], 16 * (nsub // 2) * (c + 1))
                sync.dma_start(out=ow[:, :half], in_=stage[b][:, :half]).then_inc(
                    ssA[b], 16
                )
                sync.wait_ge(gsB[b], 16 * (nsub - nsub // 2) * (c + 1))
                sync.dma_start(out=ow[:, half:], in_=stage[b][:, half:]).then_inc(
                    ssB[b], 16
                )

        @block.gpsimd
        def _(gpsimd):
            from concourse import library_config

            gpsimd.load_library(library_config.mlp)
            gpsimd.wait_ge(ls, 16)
            rsub = gpsimd.to_reg(SUB)
            for s in range(NSEG):
                b, c = s % 2, s // 2
                lo = s * SEG
                hi = min((s + 1) * SEG, VOCAB)
                for j in range(nsub):
                    if c >= 1 and j == 0:
                        gpsimd.wait_ge(ssA[b], 16 * c)
                    if c >= 1 and j == nsub // 2:
                        gpsimd.wait_ge(ssB[b], 16 * c)
                    gpsimd.dma_gather(
                        out_ap=stage[b][
                            :, j * sub_slots * D : (j + 1) * sub_slots * D
                        ].rearrange("p (s d) -> p s d", s=sub_slots, d=D),
                        in_ap=table[lo:hi, :],
                        idxs_ap=t_idx[
                            :, s * cw + j * (SUB // 16) : s * cw + (j + 1) * (SUB // 16)
                        ],
                        num_idxs=SUB,
                        num_idxs_reg=rsub,
                        elem_size=D,
                        queue_num=j % 4,
                    ).then_inc(gsA[b] if j < nsub // 2 else gsB[b], 16)

    nc.compile()
    return nc


def _pack_core(vals, cap):
    """Sort one core's lookups, bucket into NSEG int16 windows of `cap` slots.

    Returns (idx16 [P, NSEG*cap//16] int16, devrow [FLAT]: for each original
    position, the device output row holding its embedding)."""
    order = np.argsort(vals, kind="stable")
    sv = vals[order]
    bounds = np.searchsorted(sv, np.arange(NSEG + 1) * SEG)
    counts = np.diff(bounds)
    if counts.max() > cap:
        raise OverflowError(int(counts.max()))
    locals16 = (sv & (SEG - 1)).astype(np.int16)
    packed = np.zeros((NSEG, cap), np.int16)  # filler 0 = row s*SEG, harmless
    win = np.arange(FLAT) - bounds[:-1].repeat(counts)  # within-window pos
    seg_of = np.repeat(np.arange(NSEG), counts)
    packed[seg_of, win] = locals16
    # idx i of each SUB-chunk -> [i%16, i//16], replicated across the 8
    # partition groups
    blk = packed.reshape(NSEG * (cap // SUB), SUB // 16, 16)
    idx16 = np.tile(
        blk.transpose(2, 0, 1).reshape(1, 16, -1), (8, 1, 1)
    ).reshape(P, -1)
    # window pos w = j*SUB + i: stage col = j*(SUB//P) + i//P, partition i%P;
    # store is partition-major: device row = s*cap + part*slots + col
    slots = cap // P
    j, i = win // SUB, win % SUB
    devrow_sorted = seg_of * cap + (i % P) * slots + j * (SUB // P) + i // P
    devrow = np.empty(FLAT, np.int64)
    devrow[order] = devrow_sorted
    return idx16, devrow


def run(indices, table, dummy=None, trace=False, cap=CAP):
    from concourse.bass_utils import run_bass_kernel_spmd

    flat = np.asarray(indices).reshape(NCORES, FLAT).astype(np.int32)
    tab = np.ascontiguousarray(np.asarray(table), dtype=np.float32)

    while True:
        try:
            packs = [_pack_core(flat[c], cap) for c in range(NCORES)]
            break
        except OverflowError as e:
            cap = ((e.args[0] + SUB - 1) // SUB + 1) * SUB  # round up, retry

    if cap not in _built:
        _built[cap] = _build(cap)
    nc = _built[cap]

    in_maps = [{"idx16": packs[c][0], "table": tab} for c in range(NCORES)]
    kres = run_bass_kernel_spmd(nc, in_maps, list(range(NCORES)), trace=trace)
    out = np.empty((NCORES, FLAT, D), np.float32)
    for c in range(NCORES):
        dev = kres.results[c]["out"]
        out[c] = dev[packs[c][1]]
    return out.reshape(BATCH, HIST, D), kres


def kernel(indices, table, dummy=None):
    return run(indices, table, dummy)[0]
